# revision 1
# baseline (speedup 1.0000x reference)
"""CSC (class-specific context) forward — Bass/Tile kernel for 8 TRN2 cores.

Contract: kernel(cpgs, labels, preds, rois) -> (W [4000,20] f32, PL, NL),
matching reference._csc_forward. Accepts FULL inputs, shards internally:
8 cores = 4 images x 2 class-halves (data-parallel, no cross-device traffic).

Per core (image n, 10-class block):
  1. FG-threshold the CPG maps (exact: max-reduce -> 0.1*max -> compare*mult).
  2. Exact fp32 integral images: sequential y-scan and x-scan via DVE
     tensor_tensor_scan, with bitwise-exact PE transposes between them.
  3. Write a class-interleaved integral image ii2[(y,x), c] to DRAM
     (+ a zero row at index 0 for clipped corners).
  4. Gather 8 box corners per ROI (indirect DMA, one row of 10 classes per
     descriptor); ROIs are pre-sorted by context-bottom y so early chunks
     depend only on an ii2 prefix and overlap the map phase.
  5. Per-(roi, class) CSC math: box sums, score, per-image pos/neg
     normalization, mass gating, active gating.

Host side mirrors the reference's ROI geometry bit-exactly (float32) and
computes corner row indices + per-ROI reciprocals; outputs are scattered back
by the saved permutation.
"""
import json
import numpy as np

# ---------------- problem constants ----------------
N_IMG, N_CLS, H, W, N_ROI = 4, 20, 512, 512, 4000
TAU, FG_T, MASS_T, CTX = 0.7, 0.1, 0.2, 1.8
EPS = 1e-12
C = 10                      # classes per core
NROWS = H * W + 1           # ii2 rows (+ zero row at index 0)

_WS_INSTALLED = False
_KERNEL_CACHE = {}


def _install_waitsplit():
    """This walrus build accepts at most ONE semaphore wait per instruction.
    Split every multi-wait BIR instruction into single-wait NoOps + the op."""
    global _WS_INSTALLED
    if _WS_INSTALLED:
        return
    _WS_INSTALLED = True
    import concourse.bass_utils as bu
    import concourse.bass2jax as b2j

    orig = bu.compile_bir_kernel

    def split_multi_waits(bir_bytes):
        m = json.loads(bir_bytes)
        ctr = 0
        for fn in m.get("functions", []):
            for blk in fn.get("blocks", []):
                new = []
                for inst in blk["instructions"]:
                    si = inst.get("sync_info")
                    eng = inst.get("engine")
                    if si and eng:
                        waits = si.get("on_wait") or []
                        if len(waits) > 1:
                            for w in waits[:-1]:
                                ctr += 1
                                nop = {
                                    "engine": eng,
                                    "ins": [],
                                    "name": f"I-ws{ctr}",
                                    "opcode": "NoOp",
                                    "outs": [],
                                    "sync_info": {"on_update": [], "on_wait": [w]},
                                    "text_hint": "waitsplit",
                                }
                                if inst.get("debug") is not None:
                                    nop["debug"] = inst["debug"]
                                new.append(nop)
                            si["on_wait"] = [waits[-1]]
                    new.append(inst)
                blk["instructions"] = new
        return json.dumps(m).encode()

    def patched(bir_json, tmpdir, neff_name="file.neff"):
        if isinstance(bir_json, str):
            bir_json = bir_json.encode()
        return orig(split_multi_waits(bir_json), tmpdir, neff_name)

    bu.compile_bir_kernel = patched
    b2j.compile_bir_kernel = patched


def _build_kernel(nch, qbnd):
    import concourse.bass as bass
    import concourse.tile as tile
    import concourse.mybir as mybir
    from concourse import masks
    import contextlib

    F32 = mybir.dt.float32
    I32 = mybir.dt.int32
    I8 = mybir.dt.int8
    AL = mybir.AluOpType

    def _ap(t, offset, dims):
        return bass.AP(tensor=t.tensor, offset=t.offset + offset, ap=dims)

    nc = bass.Bass()
    cpg = nc.dram_tensor("cpg", [C, H, W], F32, kind="ExternalInput")
    offs_in = nc.dram_tensor("offs", [128, nch * 8], I32, kind="ExternalInput")
    rin_in = nc.dram_tensor("rin", [128, nch * C], F32, kind="ExternalInput")
    rfr_in = nc.dram_tensor("rfr", [128, nch * C], F32, kind="ExternalInput")
    act_in = nc.dram_tensor("act", [128, nch * C], I8, kind="ExternalInput")
    w_out = nc.dram_tensor("wout", [128, nch, C], F32, kind="ExternalOutput")
    ii2 = nc.dram_tensor("ii2", [NROWS, C], F32, kind="Internal")

    with tile.TileContext(nc) as tc, contextlib.ExitStack() as ctx:
        consts = ctx.enter_context(tc.tile_pool(name="consts", bufs=1))
        big = ctx.enter_context(tc.tile_pool(name="big", bufs=1))
        mcp = ctx.enter_context(tc.tile_pool(name="mcp", bufs=2))
        mtp = ctx.enter_context(tc.tile_pool(name="mtp", bufs=2))
        iiyp = ctx.enter_context(tc.tile_pool(name="iiyp", bufs=2))
        smal = ctx.enter_context(tc.tile_pool(name="smal", bufs=4))
        psA = ctx.enter_context(tc.tile_pool(name="psA", bufs=2, space="PSUM"))
        psB = ctx.enter_context(tc.tile_pool(name="psB", bufs=3, space="PSUM"))
        psS = ctx.enter_context(tc.tile_pool(name="psS", bufs=1, space="PSUM"))
        roip = ctx.enter_context(tc.tile_pool(name="roip", bufs=1))

        ident = consts.tile([128, 128], F32)
        masks.make_identity(nc, ident[:])
        ones = consts.tile([128, 512], F32)
        nc.vector.memset(ones[:], 1.0)
        zer = consts.tile([1, 16], F32)
        nc.vector.memset(zer[:], 0.0)
        nc.sync.dma_start(out=_ap(ii2.ap(), 0, [[C, 1], [1, C]]), in_=zer[:1, :C])

        offs_sb = roip.tile([128, nch * 8], I32)
        nc.sync.dma_start(out=offs_sb[:], in_=offs_in.ap())
        JC = nch * C
        rin90 = roip.tile([128, JC], F32)
        nc.sync.dma_start(out=rin90[:], in_=rin_in.ap())
        rfr90 = roip.tile([128, JC], F32)
        nc.sync.dma_start(out=rfr90[:], in_=rfr_in.ap())
        act90 = roip.tile([128, JC], I8)
        nc.sync.dma_start(out=act90[:], in_=act_in.ap())

        staging = [big.tile([128, 512 * C], F32, tag=f"stg{yb}", name=f"stg{yb}")
                   for yb in range(4)]

        for c in range(C):
            mc = mcp.tile([128, 4, 512], F32)
            nc.sync.dma_start(
                out=mc[:],
                in_=_ap(cpg.ap(), c * H * W, [[512, 128], [65536, 4], [1, 512]]),
            )
            mc2 = mc.rearrange("p a b -> p (a b)")
            pmax = smal.tile([128, 1], F32)
            nc.vector.tensor_reduce(
                out=pmax[:], in_=mc2, axis=mybir.AxisListType.X, op=AL.max
            )
            # all-partition max: PE-transpose of the free-broadcast (exact),
            # then per-partition reduce and 0.1x (exact)
            pm2 = psS.tile([128, 128], F32)
            p1 = pmax[:]
            nc.tensor.transpose(
                out=pm2[:], in_=_ap(p1, 0, [p1.ap[0], [0, 128]]), identity=ident[:]
            )
            tbc = smal.tile([128, 1], F32)
            nc.vector.tensor_reduce(
                out=tbc[:], in_=pm2[:], axis=mybir.AxisListType.X, op=AL.max
            )
            nc.vector.tensor_scalar_mul(tbc[:], tbc[:], FG_T)

            mth = mtp.tile([128, 4, 512], F32)
            nc.vector.scalar_tensor_tensor(
                out=mth.rearrange("p a b -> p (a b)"),
                in0=mc2,
                scalar=tbc[:],
                in1=mc2,
                op0=AL.is_ge,
                op1=AL.mult,
            )
            iiyT = iiyp.tile([128, 4, 512], F32)
            for xb in range(4):
                pa = psA.tile([128, 512], F32)
                for yo in range(4):
                    nc.tensor.transpose(
                        out=pa[:, yo * 128:(yo + 1) * 128],
                        in_=mth[:, yo, xb * 128:(xb + 1) * 128],
                        identity=ident[:],
                    )
                nc.vector.tensor_tensor_scan(
                    out=iiyT[:, xb, :],
                    data0=ones[:],
                    data1=pa[:],
                    initial=0.0,
                    op0=AL.mult,
                    op1=AL.add,
                )
            for yb in range(4):
                pb = psB.tile([128, 512], F32)
                for xb in range(4):
                    nc.tensor.transpose(
                        out=pb[:, xb * 128:(xb + 1) * 128],
                        in_=iiyT[:, xb, yb * 128:(yb + 1) * 128],
                        identity=ident[:],
                    )
                stg = staging[yb]
                nc.vector.tensor_tensor_scan(
                    out=_ap(stg[:], c, [stg[:].ap[0], [C, 512]]),
                    data0=ones[:],
                    data1=pb[:],
                    initial=0.0,
                    op0=AL.mult,
                    op1=AL.add,
                )

        for yb in range(4):
            nc.sync.dma_start(
                out=_ap(ii2.ap(), C + yb * 128 * 512 * C,
                        [[512 * C, 128], [1, 512 * C]]),
                in_=staging[yb][:],
            )

        # totals (full-map sums) -> mass-gate threshold, broadcast via the
        # exact transpose trick
        tot = smal.tile([1, C], F32)
        nc.sync.dma_start(
            out=tot[:], in_=_ap(ii2.ap(), (H * W) * C, [[C, 1], [1, C]])
        )
        totm = smal.tile([1, C], F32)
        nc.vector.tensor_scalar(totm[:], tot[:], EPS, MASS_T, op0=AL.max, op1=AL.mult)
        # replicate [1, C] threshold row to all partitions: PE transpose of a
        # partition-broadcast is unavailable (K=1), so use a DRAM roundtrip
        bcd = nc.dram_tensor("bcd", [1, 512], F32, kind="Internal")
        for j in range(nch):
            nc.sync.dma_start(
                out=_ap(bcd.ap(), 32 + j * C, [[1, 1], [1, C]]), in_=totm[:]
            )
        th90 = roip.tile([128, nch * C], F32)
        nc.sync.dma_start(out=th90[:], in_=_ap(bcd.ap(), 32, [[0, 128], [1, nch * C]]))

        # corner gathers (rois host-sorted by context-bottom y; early chunks
        # read only an ii2 prefix so region deps overlap the map phase)
        G = roip.tile([128, 8, nch * C], F32)
        for j in range(nch):
            nrow = 1 + qbnd[j] * 128 * 512
            for t in range(8):
                k = t * nch + j
                nc.gpsimd.indirect_dma_start(
                    out=G[:, t, j * C:(j + 1) * C],
                    out_offset=None,
                    in_=ii2.ap()[0:nrow, :],
                    in_offset=bass.IndirectOffsetOnAxis(
                        ap=offs_sb[:, k:k + 1], axis=0
                    ),
                )

        def gview(t):
            return G[:, t, :]

        s_in = roip.tile([128, JC], F32)
        nc.vector.tensor_tensor(s_in[:], gview(0), gview(1), op=AL.subtract)
        nc.vector.tensor_tensor(s_in[:], s_in[:], gview(2), op=AL.subtract)
        nc.vector.tensor_tensor(s_in[:], s_in[:], gview(3), op=AL.add)
        s_fr = roip.tile([128, JC], F32)
        nc.vector.tensor_tensor(s_fr[:], gview(4), gview(5), op=AL.subtract)
        nc.vector.tensor_tensor(s_fr[:], s_fr[:], gview(6), op=AL.subtract)
        nc.vector.tensor_tensor(s_fr[:], s_fr[:], gview(7), op=AL.add)
        nc.vector.tensor_tensor(s_fr[:], s_fr[:], s_in[:], op=AL.subtract)

        q1 = roip.tile([128, JC], F32)
        nc.vector.tensor_tensor(q1[:], s_in[:], rin90[:], op=AL.mult)
        q2 = roip.tile([128, JC], F32)
        nc.vector.tensor_tensor(q2[:], s_fr[:], rfr90[:], op=AL.mult)
        score = roip.tile([128, JC], F32)
        nc.vector.tensor_tensor(score[:], q1[:], q2[:], op=AL.subtract)

        vmask = roip.tile([128, JC], F32)
        nc.vector.tensor_tensor(vmask[:], s_in[:], th90[:], op=AL.is_ge)

        # pos/neg per class: per-partition reduce over chunks, exact
        # transpose, cross-partition reduce (32-aligned blocks)
        pn = roip.tile([128, 64], F32)
        nc.vector.memset(pn[:], 0.0)
        sc0 = score[:]
        score_cj = _ap(sc0, 0, [sc0.ap[0], [1, C], [C, nch]])
        nc.vector.tensor_reduce(
            out=pn[:, 0:C], in_=score_cj, axis=mybir.AxisListType.X, op=AL.max
        )
        nc.vector.tensor_reduce(
            out=pn[:, 32:32 + C], in_=score_cj, axis=mybir.AxisListType.X, op=AL.min
        )
        pnt = psS.tile([64, 128], F32)
        nc.tensor.transpose(out=pnt[:], in_=pn[:], identity=ident[:])
        pn1 = smal.tile([64, 1], F32)
        nc.vector.tensor_reduce(
            out=pn1[:], in_=pnt[:], axis=mybir.AxisListType.X, op=AL.max
        )
        nc.vector.tensor_reduce(
            out=pn1[32:64, :], in_=pnt[32:64, :], axis=mybir.AxisListType.X, op=AL.min
        )
        nrm = smal.tile([64, 1], F32)
        nc.vector.tensor_scalar_max(nrm[0:32, :], pn1[0:32, :], EPS)
        nc.vector.tensor_scalar(
            nrm[32:64, :], pn1[32:64, :], -1.0, EPS, op0=AL.mult, op1=AL.max
        )
        rnrm = smal.tile([64, 1], F32)
        nc.vector.reciprocal(rnrm[:], nrm[:])
        for j in range(nch):
            nc.sync.dma_start(
                out=_ap(bcd.ap(), 128 + j * C, [[1, C], [1, 1]]), in_=rnrm[0:C, :]
            )
            nc.sync.dma_start(
                out=_ap(bcd.ap(), 256 + j * C, [[1, C], [1, 1]]),
                in_=rnrm[32:32 + C, :],
            )
        rpos90 = roip.tile([128, JC], F32)
        nc.sync.dma_start(out=rpos90[:], in_=_ap(bcd.ap(), 128, [[0, 128], [1, JC]]))
        rneg90 = roip.tile([128, JC], F32)
        nc.sync.dma_start(out=rneg90[:], in_=_ap(bcd.ap(), 256, [[0, 128], [1, JC]]))

        msk = roip.tile([128, JC], I8)
        nc.vector.tensor_scalar(msk[:], score[:], 0.0, None, op0=AL.is_ge)
        rsel = roip.tile([128, JC], F32)
        nc.vector.tensor_copy(rsel[:], rneg90[:])
        nc.vector.copy_predicated(rsel[:], msk[:], rpos90[:])
        wv = roip.tile([128, JC], F32)
        nc.vector.tensor_tensor(wv[:], score[:], rsel[:], op=AL.mult)
        nc.vector.tensor_tensor(wv[:], wv[:], vmask[:], op=AL.mult)
        nc.vector.tensor_scalar_min(wv[:], wv[:], 1.0)
        nc.vector.tensor_scalar_max(wv[:], wv[:], -1.0)

        Wt = roip.tile([128, JC], F32)
        nc.vector.memset(Wt[:], 1.0)
        nc.vector.copy_predicated(Wt[:], act90[:], wv[:])
        nc.sync.dma_start(out=w_out.ap().rearrange("p a b -> p (a b)"), in_=Wt[:])

    return nc


def _host_prep(cpgs, labels, preds, rois, nch, qbnd):
    f32 = np.float32
    b = rois[:, 0].astype(np.int32)
    x1 = np.clip(np.round(rois[:, 1]), 0, W - 1).astype(np.int32)
    y1 = np.clip(np.round(rois[:, 2]), 0, H - 1).astype(np.int32)
    x2 = np.clip(np.round(rois[:, 3]), 0, W - 1).astype(np.int32)
    y2 = np.clip(np.round(rois[:, 4]), 0, H - 1).astype(np.int32)
    cx = (x1 + x2).astype(f32) * f32(0.5)
    cy = (y1 + y2).astype(f32) * f32(0.5)
    hw = (x2 - x1 + 1).astype(f32) * f32(CTX) * f32(0.5)
    hh = (y2 - y1 + 1).astype(f32) * f32(CTX) * f32(0.5)
    cx1 = np.clip(np.round(cx - hw), 0, W - 1).astype(np.int32)
    cy1 = np.clip(np.round(cy - hh), 0, H - 1).astype(np.int32)
    cx2 = np.clip(np.round(cx + hw), 0, W - 1).astype(np.int32)
    cy2 = np.clip(np.round(cy + hh), 0, H - 1).astype(np.int32)
    a_in = ((x2 - x1 + 1) * (y2 - y1 + 1)).astype(f32)
    a_ctx = ((cx2 - cx1 + 1) * (cy2 - cy1 + 1)).astype(f32)
    a_fr = np.maximum(a_ctx - a_in, f32(1.0))
    rin = (f32(1.0) / np.sqrt(a_in)).astype(f32)
    rfr = (f32(1.0) / np.sqrt(a_fr)).astype(f32)

    corn_y = np.stack([y2 + 1, y1, y2 + 1, y1, cy2 + 1, cy1, cy2 + 1, cy1], axis=1)
    corn_x = np.stack([x2 + 1, x2 + 1, x1, x1, cx2 + 1, cx2 + 1, cx1, cx1], axis=1)
    idx = 1 + (corn_y - 1) * W + (corn_x - 1)
    idx = np.where((corn_y == 0) | (corn_x == 0), 0, idx).astype(np.int32)

    active = ((labels >= 0.5) & (preds >= f32(TAU))).astype(np.int8)

    rpad = nch * 128
    in_maps, meta = [], []
    ok = True
    for core in range(8):
        n, h = core // 2, core % 2
        sel = np.where(b == n)[0]
        rn = len(sel)
        if rn > rpad:
            ok = False
            break
        if rn:
            sel = sel[np.argsort(cy2[sel], kind="stable")]
        pad = np.concatenate([sel, np.full(rpad - rn, sel[0] if rn else 0, np.int64)])
        pm = pad.reshape(nch, 128)  # roi i = j*128 + p
        mx = idx[pm].max(axis=(1, 2))
        for j in range(nch):
            if mx[j] > qbnd[j] * 128 * 512:
                ok = False
        if not ok:
            break
        offs = np.transpose(idx[pm], (1, 2, 0)).reshape(128, 8 * nch).astype(np.int32)
        in_maps.append(
            {
                "cpg": np.ascontiguousarray(cpgs[n, h * C:(h + 1) * C]),
                "offs": np.ascontiguousarray(offs),
                "rin": np.ascontiguousarray(
                    np.broadcast_to(rin[pm].T[:, :, None], (128, nch, C))
                ).reshape(128, nch * C),
                "rfr": np.ascontiguousarray(
                    np.broadcast_to(rfr[pm].T[:, :, None], (128, nch, C))
                ).reshape(128, nch * C),
                "act": np.ascontiguousarray(
                    np.broadcast_to(
                        active[n, h * C:(h + 1) * C][None, None, :], (128, nch, C)
                    )
                ).reshape(128, nch * C),
            }
        )
        meta.append((n, h, sel))
    return (in_maps, meta) if ok else (None, None)


def kernel(cpgs, labels, preds, rois):
    _install_waitsplit()
    from concourse.bass_utils import run_bass_kernel_spmd

    cpgs = np.ascontiguousarray(np.asarray(cpgs, dtype=np.float32))
    labels = np.asarray(labels, dtype=np.float32)
    preds = np.asarray(preds, dtype=np.float32)
    rois = np.asarray(rois, dtype=np.float32)

    # config ladder: default tuned for the benchmark distribution; fall back
    # to conservative bounds / larger padding for unusual inputs
    configs = [
        (9, (2, 3, 4, 4, 4, 4, 4, 4, 4)),
        (9, (4, 4, 4, 4, 4, 4, 4, 4, 4)),
        (16, (4,) * 16),
        (32, (4,) * 32),
    ]
    for nch, qbnd in configs:
        prep = _host_prep(cpgs, labels, preds, rois, nch, qbnd)
        if prep[0] is not None:
            break
    else:
        raise ValueError("ROI distribution exceeds all kernel configurations")
    in_maps, meta = prep

    key = (nch, qbnd)
    if key not in _KERNEL_CACHE:
        _KERNEL_CACHE[key] = _build_kernel(nch, qbnd)
    nc = _KERNEL_CACHE[key]

    res = run_bass_kernel_spmd(nc, in_maps, core_ids=list(range(8)))

    Wf = np.empty((N_ROI, N_CLS), np.float32)
    for core, (n, h, sel) in enumerate(meta):
        out = np.asarray(res.results[core]["wout"]).reshape(128, nch, C)
        rn = len(sel)
        if rn == 0:
            continue
        flat = np.transpose(out, (1, 0, 2)).reshape(nch * 128, C)
        Wf[sel, h * C:(h + 1) * C] = flat[:rn]
    PL = labels.copy()
    NL = np.zeros_like(PL)
    return Wf, PL, NL
